# revision 30
# baseline (speedup 1.0000x reference)
"""Sliding-window attention (window=256) for B=4, S=2048, D=1024, H=16, HD=64
on 8 Trainium2 NeuronCores.

Sharding: core c handles batch b = c//2, sequence half h = c%2 (1024 query
tokens). K/V are computed for the core's own tokens plus a 256-token halo
from the previous chunk (zero-padded for h=0), so no cross-core communication
is needed.

All matmuls run in bf16 (fp32 PSUM accumulate) so FWL hides weight loads,
and element-wise ops run on 2-byte data for 2x DVE throughput.

Software-pipelined structure (everything is in-order per engine, so emission
order is schedule order):
  - inputs stream per contraction block k: the first projection matmul only
    waits for the first 1/8th of the weight/activation DMAs.
  - Q-projection m-blocks are interleaved with V-projection kb-blocks so the
    V matmuls fill the PE bubbles left by the RoPE chain.
  - the RoPE pair-swap matmul for block m is emitted after the projection
    matmuls of block m+1 (one-stage software pipeline).
  - in attention, the attn@V/reciprocal/normalize stage for iteration i is
    emitted after the scores/exp/mask stage of iteration i+1, and the output
    projections of query group g are deferred into group g+1's loop.
  - the attention@V stationary carries 64 replicated ones-columns so the
    softmax denominator lands pre-broadcast in PSUM partitions 64..127
    (no gpsimd partition_broadcast).
  - the two heads of a pair are emitted interleaved at tile_position row
    halves 0/64 so their score matmuls stream concurrently through the two
    halves of the PE array.
"""

import sys

for _p in ("/opt/trn_rl_repo", "/root/.axon_site/_ro/trn_rl_repo"):
    if _p not in sys.path:
        sys.path.insert(0, _p)

import numpy as np
import ml_dtypes

import concourse.bacc as bacc
import concourse.mybir as mybir
from concourse.tile import TileContext
from concourse.bass_utils import run_bass_kernel_spmd

f32 = mybir.dt.float32
bf16 = mybir.dt.bfloat16
AF = mybir.ActivationFunctionType
OP = mybir.AluOpType

B, S, D = 4, 2048, 1024
H, HD = 16, 64
WINDOW = 256
NT = 1024          # query tokens per core
NKV = 1280         # extended kv tokens per core (256 halo + 1024)
NB = 8             # d blocks of 128
KBLK = 10          # kv token blocks of 128
N_CORES = 8

_NC_CACHE = None


def _emit(nc, reps=1):
    XT = nc.dram_tensor("XT", [8, 128, NKV], bf16, kind="ExternalInput")
    WQT = nc.dram_tensor("WQT", [8, 128, D], bf16, kind="ExternalInput")
    WKT = nc.dram_tensor("WKT", [8, 128, D], bf16, kind="ExternalInput")
    WVT = nc.dram_tensor("WVT", [8, 128, D], bf16, kind="ExternalInput")
    WOT = nc.dram_tensor("WOT", [8, 128, D], bf16, kind="ExternalInput")
    COSQ = nc.dram_tensor("COSQ", [128, NT], bf16, kind="ExternalInput")
    SINQ = nc.dram_tensor("SINQ", [128, NT], bf16, kind="ExternalInput")
    COSKV = nc.dram_tensor("COSKV", [128, NKV], bf16, kind="ExternalInput")
    SINKV = nc.dram_tensor("SINKV", [128, NKV], bf16, kind="ExternalInput")
    ONESC = nc.dram_tensor("ONESC", [128, 160, 64], bf16, kind="ExternalInput")
    MASKS = nc.dram_tensor("MASKS", [128, 1024], bf16, kind="ExternalInput")
    OUT = nc.dram_tensor("OUT", [NT, D], f32, kind="ExternalOutput")

    with TileContext(nc, pool_alloc_mode="queue") as tc:
      for _rep in range(reps):
        with (
            tc.tile_pool(name="consts", bufs=1) as p_const,
            tc.tile_pool(name="qkres", bufs=1) as p_qk,
            tc.tile_pool(name="vres", bufs=1) as p_v,
        ):
            qrot = [p_qk.tile([128, NT], bf16, name=f"qrot{m}") for m in range(NB)]
            krot = [p_qk.tile([128, NKV], bf16, name=f"krot{m}") for m in range(NB)]
            v_sb = p_v.tile([128, KBLK * 16, 128], bf16, name="vsb")

            with tc.tile_pool(name="xres", bufs=1) as p_x:
                with (
                    tc.tile_pool(name="wq", bufs=1) as p_wq,
                    tc.tile_pool(name="wk", bufs=1) as p_wk,
                    tc.tile_pool(name="wvstream", bufs=1) as p_wv,
                    tc.tile_pool(name="ropetmp", bufs=2) as p_t,
                ):
                    # stream inputs per contraction block: first matmul waits
                    # only for the k=0 slices
                    xkv = p_x.tile([128, 8, NKV], bf16)
                    wq_sb = p_wq.tile([128, 8, D], bf16)
                    for k in range(8):
                        nc.sync.dma_start(wq_sb[:, k, :], WQT[k])
                        nc.sync.dma_start(xkv[:, k, :], XT[k])
                    wvs = []
                    for dch in range(2):
                        wv = p_wv.tile([128, 8, 512], bf16, tag=f"wv{dch}",
                                       name=f"wv{dch}")
                        nc.sync.dma_start(
                            wv[:],
                            WVT[:, :, dch * 512 : (dch + 1) * 512].rearrange(
                                "kb p d -> p kb d"
                            ),
                        )
                        wvs.append(wv)
                    wk_sb = p_wk.tile([128, 8, D], bf16)
                    for k in range(8):
                        nc.sync.dma_start(wk_sb[:, k, :], WKT[k])
                    nc.sync.dma_start(v_sb[:, :, 64:128], ONESC[:])
                    masks = p_const.tile([128, 1024], bf16)
                    nc.sync.dma_start(masks[:], MASKS[:])
                    # trigger the exp table load off the critical path
                    warm = p_t.tile([1, 8], bf16, tag="warm", name="warm")
                    nc.scalar.activation(warm[:], masks[0:1, 0:8],
                                         AF.Exp, bias=0.0, scale=1.0)

                    # ---- Phase 1+2: projections + RoPE, V interleaved -------
                    def v_proj(kb, p_pv):
                        # dch-separated: dch0's psum copy overlaps dch1's MMs
                        for dch in range(2):
                            pv = p_pv.tile([128, 512], f32, tag=f"pv{dch}",
                                           bufs=2, name=f"pv{dch}")
                            for k in range(8):
                                nc.tensor.matmul(
                                    pv[:],
                                    xkv[:, k, kb * 128 : (kb + 1) * 128],
                                    wvs[dch][:, k, :],
                                    start=(k == 0),
                                    stop=(k == 7),
                                )
                            nc.scalar.copy(
                                v_sb[:, kb * 16 + 8 * dch :
                                     kb * 16 + 8 * dch + 8, 0:64],
                                pv[:].rearrange("p (h e) -> p h e", e=64),
                            )

                    def rope_tail(job):
                        # pair-swap via vector-engine stream shuffle
                        # (p XOR 16 in the deinterleaved head-dim layout)
                        # + cos/sin combine, for an earlier m-block
                        rot_t, m, items, cos_t, sin_t = job
                        for ci, off, cw, raw in items:
                            sh = p_t.tile([128, 512], bf16, tag=f"sh_{ci}",
                                          name="sh")
                            nc.vector.stream_shuffle(
                                sh[:, :cw], raw[:, :cw],
                                mask=[i ^ 16 for i in range(32)],
                            )
                            t1 = p_t.tile([128, 512], bf16, tag=f"t1_{ci}",
                                          name="t1")
                            nc.vector.tensor_mul(
                                t1[:, :cw], raw[:, :cw],
                                cos_t[:, off : off + cw]
                            )
                            t2 = p_t.tile([128, 512], bf16, tag=f"t2_{ci}",
                                          name="t2")
                            nc.vector.tensor_mul(
                                t2[:, :cw], sh[:, :cw],
                                sin_t[:, off : off + cw]
                            )
                            nc.vector.tensor_add(
                                rot_t[m][:, off : off + cw],
                                t1[:, :cw], t2[:, :cw],
                            )

                    # V kb-blocks interleaved into the Q m-loop
                    v_sched = [[], [], [0, 1], [2, 3], [4, 5], [6, 7],
                               [8], [9]]
                    specs = [
                        (wq_sb, COSQ, SINQ, NT, qrot,
                         [(0, 512), (512, 512)], 256, v_sched),
                        (wk_sb, COSKV, SINKV, NKV, krot,
                         [(0, 512), (512, 512), (1024, 256)], 0,
                         [[] for _ in range(NB)]),
                    ]
                    with (
                        tc.tile_pool(name="pproj", bufs=2, space="PSUM")
                        as p_ps,
                        tc.tile_pool(name="pvp", bufs=1, space="PSUM")
                        as p_pv,
                    ):
                     for (w_sb, COS_d, SIN_d, ncols, rot_t, chunks, colbase,
                          vsch) in specs:
                      with tc.tile_pool(name="rope_cs", bufs=1) as p_cs:
                        cos_t = p_cs.tile([128, ncols], bf16, name="cos_t")
                        nc.sync.dma_start(cos_t[:], COS_d[:])
                        sin_t = p_cs.tile([128, ncols], bf16, name="sin_t")
                        nc.sync.dma_start(sin_t[:], SIN_d[:])
                        pend = None
                        for m in range(NB):
                            psums = [
                                (p_ps.tile([128, 512], f32, tag=f"pp{ci}",
                                           name=f"pp{ci}")
                                 if ci < 2 else
                                 p_pv.tile([128, 512], f32, tag="pv0",
                                           bufs=2, name="pp2"))
                                for ci in range(len(chunks))
                            ]
                            for k in range(8):
                                for ci, (off, cw) in enumerate(chunks):
                                    nc.tensor.matmul(
                                        psums[ci][:, :cw],
                                        w_sb[:, k, m * 128 : (m + 1) * 128],
                                        xkv[:, k, colbase + off : colbase + off + cw],
                                        start=(k == 0),
                                        stop=(k == 7),
                                    )
                            items = []
                            for ci, (off, cw) in enumerate(chunks):
                                raw = p_t.tile([128, 512], bf16,
                                               tag=f"raw{ci}", name="raw")
                                nc.scalar.copy(raw[:, :cw], psums[ci][:, :cw])
                                items.append((ci, off, cw, raw))
                            if pend is not None:
                                rope_tail(pend)
                            pend = (rot_t, m, items, cos_t, sin_t)
                            for kb in vsch[m]:
                                v_proj(kb, p_pv)
                        rope_tail(pend)

            # ---- Phase 3: banded attention ----------------------------------
            with tc.tile_pool(name="aores", bufs=1) as p_ao:
                attn_out = [p_ao.tile([128, NT], bf16, name=f"ao{m}")
                            for m in range(NB)]
                with (
                    tc.tile_pool(name="atmp", bufs=3) as p_at,
                    tc.tile_pool(name="wostream", bufs=1) as p_wo,
                    tc.tile_pool(name="otmp", bufs=2) as p_ot,
                    tc.tile_pool(name="psc", bufs=1, space="PSUM") as p_psc,
                    tc.tile_pool(name="pav", bufs=2, space="PSUM") as p_pav,
                    tc.tile_pool(name="po", bufs=1, space="PSUM") as p_po,
                ):
                    wo0 = p_wo.tile([128, 8, 512], bf16, tag="wo", name="wo0")
                    nc.sync.dma_start(
                        wo0[:], WOT[:, :, 0:512].rearrange("kb p d -> p kb d"))
                    wo1 = p_wo.tile([128, 8, 512], bf16, tag="wo2", name="wo1")
                    nc.sync.dma_start(
                        wo1[:], WOT[:, :, 512:1024].rearrange("kb p d -> p kb d"))

                    def emit_scores(qg, hp):
                        # scores + exp + masks; returns tiles for the tail
                        blk = hp
                        psc_a = p_psc.tile([128, 4, 256], f32, tag="psca",
                                           name="psca")
                        psc_b = p_psc.tile([128, 4, 256], f32, tag="pscb",
                                           name="pscb")
                        # chunk-outer / head-inner: the two heads stream
                        # through disjoint PE row halves concurrently
                        for c in range(4):
                            kb = 2 * qg + c
                            dst = psc_a if c < 2 else psc_b
                            for half in range(2):
                                base = 64 * half
                                nc.tensor.matmul(
                                    dst[:, 2 * half + (c % 2), :],
                                    krot[blk][base : base + 64,
                                              kb * 128 : (kb + 1) * 128],
                                    qrot[blk][base : base + 64,
                                              qg * 256 : (qg + 1) * 256],
                                    start=True,
                                    stop=True,
                                    tile_position=(base, 0),
                                )
                        at_a = p_at.tile([128, 1024], bf16, tag="attn_a",
                                         name="at_a")
                        nc.scalar.activation(
                            at_a[:], psc_a[:].rearrange("p c q -> p (c q)"),
                            AF.Exp, bias=0.0, scale=0.125,
                        )
                        # mask on the otherwise-idle gpsimd engine; its
                        # latency is hidden by the 2-deep tail pipeline
                        nc.gpsimd.affine_select(
                            at_a[:].rearrange("p (g c q) -> p g c q",
                                              g=2, q=256),
                            at_a[:].rearrange("p (g c q) -> p g c q",
                                              g=2, q=256),
                            pattern=[[0, 2], [128, 2], [-1, 256]],
                            compare_op=OP.is_gt,
                            fill=0.0,
                            base=0,
                            channel_multiplier=1,
                        )
                        at_b = p_at.tile([128, 1024], bf16, tag="attn_b",
                                         name="at_b")
                        nc.scalar.activation(
                            at_b[:], psc_b[:].rearrange("p c q -> p (c q)"),
                            AF.Exp, bias=0.0, scale=0.125,
                        )
                        nc.vector.tensor_mul(
                            at_b[:].rearrange("p (g c q) -> p g c q",
                                              g=2, q=256),
                            at_b[:].rearrange("p (g c q) -> p g c q",
                                              g=2, q=256),
                            masks[:, None, 512:1024]
                            .rearrange("p o (c q) -> p o c q", q=256)
                            .to_broadcast([128, 2, 2, 256]),
                        )
                        return (qg, hp, at_a, at_b)

                    def emit_tail(job):
                        qg, hp, at_a, at_b = job
                        blk = hp
                        pav = p_pav.tile([128, 512], f32, tag="pav",
                                         name="pav")
                        for half in range(2):
                            h = 2 * hp + half
                            for c in range(4):
                                kb = 2 * qg + c
                                a_src = at_a if c < 2 else at_b
                                nc.tensor.matmul(
                                    pav[:, half * 256 : (half + 1) * 256],
                                    v_sb[:, kb * 16 + h, :],
                                    a_src[:, (2 * half + (c % 2)) * 256 :
                                          (2 * half + (c % 2) + 1) * 256],
                                    start=(c == 0),
                                    stop=(c == 3),
                                )
                        rb = p_at.tile([64, 512], bf16, tag="rb", name="rb")
                        with nc.allow_low_precision(reason="bf16 attention"):
                            nc.vector.reciprocal(rb[:], pav[64:128, :])
                            for half in range(2):
                                base = 64 * half
                                nc.vector.tensor_mul(
                                    attn_out[blk][base : base + 64,
                                                  qg * 256 : (qg + 1) * 256],
                                    pav[0:64,
                                        half * 256 : (half + 1) * 256],
                                    rb[0:64,
                                       half * 256 : (half + 1) * 256],
                                )

                    def out_proj(tb):
                        pos = [p_po.tile([128, 512], f32, tag=f"po{d}",
                                         name=f"po{d}")
                               for d in range(2)]
                        # dch-inner: one stationary load serves both halves
                        for k in range(8):
                            for dch, wo_t in ((0, wo0), (1, wo1)):
                                nc.tensor.matmul(
                                    pos[dch][:],
                                    attn_out[k][:, tb * 128 : (tb + 1) * 128],
                                    wo_t[:, k, :],
                                    start=(k == 0),
                                    stop=(k == 7),
                                )
                        for dch in range(2):
                            osb = p_ot.tile([128, 512], f32, tag="osb",
                                            name="osb")
                            nc.scalar.copy(osb[:], pos[dch][:])
                            nc.sync.dma_start(
                                OUT[tb * 128 : (tb + 1) * 128,
                                    dch * 512 : (dch + 1) * 512],
                                osb[:],
                            )

                    pend_tails = []
                    pend_oproj = []
                    for qg in range(4):
                        for hp in range(H // 2):
                            pend_tails.append(emit_scores(qg, hp))
                            if len(pend_tails) > 2:
                                emit_tail(pend_tails.pop(0))
                            if pend_oproj and hp in (1, 3):
                                out_proj(pend_oproj.pop(0))
                        pend_oproj += [2 * qg, 2 * qg + 1]
                        if qg == 3:
                            while pend_tails:
                                emit_tail(pend_tails.pop(0))
                            for tb in pend_oproj:
                                out_proj(tb)
                            pend_oproj = []
    return nc


def _build():
    global _NC_CACHE
    if _NC_CACHE is None:
        nc = bacc.Bacc("TRN2", target_bir_lowering=False, debug=False,
                       num_devices=N_CORES)
        _emit(nc)
        nc.compile()
        _NC_CACHE = nc
    return _NC_CACHE


def _rot_matrix():
    rt = np.zeros((128, 128), np.float32)
    for hb in range(2):
        for i in range(32):
            rt[hb * 64 + 2 * i + 1, hb * 64 + 2 * i] = -1.0
            rt[hb * 64 + 2 * i, hb * 64 + 2 * i + 1] = 1.0
    return rt


def kernel(x, freqs_cos, freqs_sin, wq, wk, wv, wo):
    bf = ml_dtypes.bfloat16
    x = np.asarray(x, np.float32)
    freqs_cos = np.asarray(freqs_cos, np.float32)
    freqs_sin = np.asarray(freqs_sin, np.float32)
    wq = np.asarray(wq, np.float32)
    wk = np.asarray(wk, np.float32)
    wv = np.asarray(wv, np.float32)
    wo = np.asarray(wo, np.float32)

    def tile_w(wT):  # [1024,1024] (d_in, d_out) -> [k, 128, 1024]
        return np.ascontiguousarray(wT.reshape(8, 128, D)).astype(bf)

    # deinterleaved head-dim layout for Q/K: within each 32-row block,
    # rows [0:16] hold even components 2f, rows [16:32] odd components
    # 2f+1 with f = 16*(block%2) + (row%16); RoPE pair-swap = row XOR 16
    p = np.arange(128)
    cperm = (p // 64) * 64 + 32 * ((p % 64) // 32) + 2 * (p % 16) \
        + ((p % 32) >= 16)
    qk_idx = (np.arange(D) // 128) * 128 + cperm[np.arange(D) % 128]

    WQT = tile_w(wq.T[:, qk_idx])
    WKT = tile_w(wk.T[:, qk_idx])
    WVT = tile_w(wv.T)
    WOT = tile_w(wo.T)
    kr = np.arange(128)[:, None]
    qr = np.arange(256)[None, :]
    m0 = (kr - qr > 0).astype(np.float32)
    m1 = (128 + kr - qr > 0).astype(np.float32)
    m2 = (qr - kr >= 0).astype(np.float32)
    m3 = (qr - kr - 128 >= 0).astype(np.float32)
    MASKS = np.concatenate([m0, m1, m2, m3], axis=1).astype(bf)

    # cos/sin expanded over partition rows for the deinterleaved layout:
    # row r -> freq index 16*((r%64)//32) + (r%16); sin carries the RoPE
    # sign (-1 for even-component rows), over extended positions -256..2047
    # (first 256 cols zero-padded).
    r = np.arange(128)
    fidx = 16 * ((r % 64) // 32) + (r % 16)
    sgn = np.where((r % 32) < 16, -1.0, 1.0).astype(np.float32)
    cos_ext = np.zeros((128, 256 + S), np.float32)
    sin_ext = np.zeros((128, 256 + S), np.float32)
    cos_ext[:, 256:] = freqs_cos[:, fidx].T
    sin_ext[:, 256:] = freqs_sin[:, fidx].T * sgn[:, None]

    in_maps = []
    for c in range(N_CORES):
        b, hh = c // 2, c % 2
        x_ext = np.zeros((NKV, D), np.float32)
        x_ext[256:] = x[b, hh * NT : (hh + 1) * NT]
        if hh == 1:
            x_ext[:256] = x[b, NT - 256 : NT]
        q0 = hh * NT
        onesc = np.ones((128, 160, 64), np.float32)
        if hh == 0:
            onesc[:, 0:32, :] = 0.0  # k-blocks 0,1 are zero-padded halo
        in_maps.append({
            "XT": np.ascontiguousarray(x_ext.T.reshape(8, 128, NKV)).astype(bf),
            "WQT": WQT, "WKT": WKT, "WVT": WVT, "WOT": WOT,
            "COSQ": np.ascontiguousarray(
                cos_ext[:, 256 + q0 : 256 + q0 + NT]).astype(bf),
            "SINQ": np.ascontiguousarray(
                sin_ext[:, 256 + q0 : 256 + q0 + NT]).astype(bf),
            "COSKV": np.ascontiguousarray(
                cos_ext[:, q0 : q0 + NKV]).astype(bf),
            "SINKV": np.ascontiguousarray(
                sin_ext[:, q0 : q0 + NKV]).astype(bf),
            "MASKS": MASKS,
            "ONESC": onesc.astype(bf),
        })

    nc = _build()
    res = run_bass_kernel_spmd(nc, in_maps, list(range(N_CORES)), trace=False)

    out = np.empty((B, S, D), np.float32)
    for c in range(N_CORES):
        b, hh = c // 2, c % 2
        out[b, hh * NT : (hh + 1) * NT] = res.results[c]["OUT"]
    return out
